# revision 1
# baseline (speedup 1.0000x reference)
"""Trainium2 Bass kernel: nn_DifferentiableSelector (soft top-K w/ refractory damping).

Data-parallel over batch: 512 rows -> 64 rows/core on 8 NeuronCores.

Device layout ("two contiguous row-chunks"): each core's [64, 32768] block is
split into 2 contiguous address-range chunks of 32 rows. Chunk k, viewed as
[128, 4096], holds rows 32k..32k+31 with row 32k+j on partitions
[4j, 4j+4) — so every DMA is one fully-contiguous 4MB transfer (measured
6-30x faster on this target than partition-interleaved patterns), and chunk
k+1's input DMA overlaps chunk k's compute while chunk k's output DMA overlaps
chunk k+1's compute. Per chunk: sigmoid as 2048-wide out-of-place ACT tiles
with fused row-partial accumulation (accum_out), one PE matmul against a 0/1
block matrix to group-sum + broadcast the row budgets, reciprocal straight
from PSUM, then one full-width (even-length, 2x-mode) DVE tensor_scalar scale
pass; column 0 of each row is then overwritten via a masked per-partition
factor to implement y[:, 0] = 0.

Math: y0 = sigmoid(scores/temp); budget_r = clip(sum_i y0[r,i], 1e-6);
y = y0 * min(K/budget, 1); then R=4 damping iters
y *= min(2/(1+y+roll(y,-d)), 1); y[:,0] = 0.

Damping-identity property (load-bearing): if budget_r >= 2K = 128 for every
row, then min(K/budget,1) <= 0.5 (correctly-rounded fp32 div), so every
y <= 0.5, so s = fl(y[i]+y[i+d]) <= 1, fl(1+s) <= 2, fl(2/(1+s)) >= 1, and
min(2/(1+s), 1.0) == 1.0 *exactly*; y*1.0 is bitwise identity. Inductively the
whole damping loop is an exact fp32 no-op. For N(0,1)-like scores,
budget ~ T/2 = 16384 (margin ~128x over the threshold). The device exports the
raw per-row sums; the host checks sum >= 256 for every row and otherwise falls
back to a full numpy evaluation of the reference semantics (exact for
arbitrary inputs; never taken for the spec'd input distribution). The same
check makes clip(budget, 1e-6) and min(K/budget, 1) identities on the device
path, so the device computes g = K * reciprocal(sum) directly.
"""

import numpy as np

B, T = 512, 32768
K = 64.0
R_REFRACTORY = 4
N_CORES = 8
ROWS = B // N_CORES  # 64 rows per core
P = 128

NCHUNK = 2
RPC = ROWS // NCHUNK  # 32 rows per chunk
GS = P // RPC  # 4 partitions per row within a chunk
WC = RPC * T // P  # 8192 free width per chunk
ACT_W = 2048  # ACT tile width

_NC_CACHE: dict = {}


def _build_nc(inv_temp: float, reps: int = 1):
    from contextlib import ExitStack

    import concourse.bacc as bacc
    import concourse.tile as tile
    from concourse import mybir

    f32 = mybir.dt.float32
    nc = bacc.Bacc(
        "TRN2",
        target_bir_lowering=False,
        debug=False,
        enable_asserts=False,
        num_devices=N_CORES,
    )
    scores_h = nc.dram_tensor("scores", [ROWS, T], f32, kind="ExternalInput")
    wsum_h = nc.dram_tensor("wsum", [P, P], f32, kind="ExternalInput")
    mask_h = nc.dram_tensor("mask", [P, 1], f32, kind="ExternalInput")
    y_h = nc.dram_tensor("y", [ROWS, T], f32, kind="ExternalOutput")
    bud_h = nc.dram_tensor("budgets", [NCHUNK, P], f32, kind="ExternalOutput")

    # [nchunk, 128, Wc] flat-contiguous chunk views
    s_k = scores_h.rearrange("r (q w) -> (r q) w", w=WC).rearrange(
        "(k p) w -> k p w", p=P
    )
    y_k = y_h.rearrange("r (q w) -> (r q) w", w=WC).rearrange("(k p) w -> k p w", p=P)

    with tile.TileContext(nc) as tc, ExitStack() as ctx:
        inp = ctx.enter_context(tc.tile_pool(name="inp", bufs=2))
        sig = ctx.enter_context(tc.tile_pool(name="sig", bufs=2))
        outp = ctx.enter_context(tc.tile_pool(name="outp", bufs=2))
        stats = ctx.enter_context(tc.tile_pool(name="stats", bufs=4))
        consts = ctx.enter_context(tc.tile_pool(name="consts", bufs=1))
        psum = ctx.enter_context(tc.tile_pool(name="psum", bufs=4, space="PSUM"))

        wsum_t = consts.tile([P, P], f32)
        nc.sync.dma_start(wsum_t[:], wsum_h[:, :])
        mask_t = consts.tile([P, 1], f32)
        nc.sync.dma_start(mask_t[:], mask_h[:, :])
        # Load the sigmoid ACT table set while the first big DMA streams.
        wtile = consts.tile([P, 1], f32)
        nc.vector.memset(wtile[:], 0.0)
        nc.scalar.activation(wtile[:], wtile[:], mybir.ActivationFunctionType.Sigmoid)

        for _rep in range(reps):
            for k in range(NCHUNK):
                t_in = inp.tile([P, WC], f32, tag="in")
                nc.sync.dma_start(t_in[:], s_k[k, :, :])
                t_sig = sig.tile([P, WC], f32, tag="sig")
                ntile = WC // ACT_W
                partials = stats.tile([P, ntile], f32, tag="partials")
                for i in range(ntile):
                    sl = slice(i * ACT_W, (i + 1) * ACT_W)
                    nc.scalar.activation(
                        t_sig[:, sl],
                        t_in[:, sl],
                        mybir.ActivationFunctionType.Sigmoid,
                        scale=float(inv_temp),
                        accum_out=partials[:, i : i + 1],
                    )
                total = stats.tile([P, 1], f32, tag="total")
                nc.vector.tensor_reduce(
                    total[:],
                    partials[:],
                    axis=mybir.AxisListType.X,
                    op=mybir.AluOpType.add,
                )
                # group-sum + broadcast: bud[p] = sum of total over p's 4-group
                bud_ps = psum.tile([P, 1], f32, tag="budps")
                nc.tensor.matmul(
                    bud_ps[:], wsum_t[:], total[:, 0:1], start=True, stop=True
                )
                rb = stats.tile([P, 1], f32, tag="rb")
                nc.vector.reciprocal(rb[:], bud_ps[:])
                gm = stats.tile([P, 1], f32, tag="gm")  # K/b with row-start zeroing
                nc.vector.tensor_scalar(
                    gm[:],
                    rb[:],
                    mask_t[:, 0:1],
                    K,
                    op0=mybir.AluOpType.mult,
                    op1=mybir.AluOpType.mult,
                )
                t_out = outp.tile([P, WC], f32, tag="out")
                # plain single-op TS with precomputed g keeps 2x mode
                g = stats.tile([P, 1], f32, tag="g")
                nc.vector.tensor_scalar_mul(g[:], rb[:], K)
                nc.vector.tensor_scalar_mul(t_out[:, :], t_sig[:, :], g[:, 0:1])
                nc.vector.tensor_mul(t_out[:, 0:1], t_sig[:, 0:1], gm[:, 0:1])
                nc.sync.dma_start(y_k[k, :, :], t_out[:])
                # export raw row sums (off critical path)
                bud = stats.tile([P, 1], f32, tag="bud")
                nc.vector.tensor_copy(bud[:], bud_ps[:])
                nc.gpsimd.dma_start(bud_h[k : k + 1, :], bud[:, 0:1])
    nc.compile()
    return nc


def _get_nc(inv_temp: float, reps: int = 1):
    key = (round(float(inv_temp), 9), reps)
    if key not in _NC_CACHE:
        _NC_CACHE[key] = _build_nc(inv_temp, reps)
    return _NC_CACHE[key]


def _wsum_matrix() -> np.ndarray:
    # wsum[k, m] = 1 iff k//GS == m//GS: sums each row's GS partitions and
    # broadcasts back to all of them — one matmul does the whole reduction.
    return np.kron(np.eye(P // GS, dtype=np.float32), np.ones((GS, GS), np.float32))


def _mask_matrix() -> np.ndarray:
    # 0 at partitions holding a row start (p % GS == 0), else 1
    m = np.ones((P, 1), np.float32)
    m[0::GS, 0] = 0.0
    return m


def _temp_from_log(log_temperature) -> np.float32:
    lt = np.float32(np.asarray(log_temperature, dtype=np.float32).reshape(()))
    return np.float32(np.clip(np.exp(lt, dtype=np.float32), 0.1, 10.0))


def _reference_fallback(scores: np.ndarray, temp: np.float32) -> np.ndarray:
    # Exact general-case evaluation (mirrors reference.py in fp32 numpy).
    y = 1.0 / (1.0 + np.exp(-(scores / temp), dtype=np.float32))
    y = y.astype(np.float32)
    budget = np.clip(np.sum(y, axis=1, keepdims=True, dtype=np.float32), 1e-6, None)
    y = y * np.minimum(np.float32(K) / budget, np.float32(1.0))
    t = scores.shape[1]
    for d in range(1, min(R_REFRACTORY + 1, t)):
        shift = np.roll(y, -d, axis=1)
        y = y * np.minimum(2.0 / (1.0 + y + shift), 1.0).astype(np.float32)
    y = y.astype(np.float32)
    y[:, 0] = 0.0
    return y


def kernel(scores: np.ndarray, log_temperature: np.ndarray) -> np.ndarray:
    from concourse.bass_utils import run_bass_kernel_spmd

    scores = np.ascontiguousarray(scores, dtype=np.float32)
    assert scores.shape == (B, T), scores.shape
    temp = _temp_from_log(log_temperature)
    inv_temp = np.float32(1.0) / temp

    nc = _get_nc(float(inv_temp))
    wsum = _wsum_matrix()
    mask = _mask_matrix()
    in_maps = [
        {"scores": scores[c * ROWS : (c + 1) * ROWS], "wsum": wsum, "mask": mask}
        for c in range(N_CORES)
    ]
    res = run_bass_kernel_spmd(nc, in_maps, list(range(N_CORES))).results
    y = np.concatenate([res[c]["y"] for c in range(N_CORES)], axis=0)
    # budgets[k, GS*j] = raw sum of row RPC*k + j (per core)
    budgets = np.concatenate(
        [res[c]["budgets"][:, 0::GS].reshape(-1) for c in range(N_CORES)]
    )

    # Damping is an exact fp32 identity iff every row budget >= 2K (see module
    # docstring); 256 adds 2x margin over the required 128. If violated (never,
    # for randn-scale inputs), recompute everything faithfully on the host.
    if not np.all(budgets >= 256.0):
        return _reference_fallback(scores, temp)
    return y



# revision 2
# speedup vs baseline: 1.1528x; 1.1528x over previous
"""Trainium2 Bass kernel: nn_DifferentiableSelector (soft top-K w/ refractory damping).

Data-parallel over batch: 512 rows -> 64 rows/core on 8 NeuronCores.

Memory-regime kernel: the only real lever is HBM bytes, so device I/O is
fp16 (half the traffic of the fp32 baseline). The host rounds scores to
fp16 (worst-case sigmoid rel-err ~|x|*2^-11 ~ 3e-3 at the |x|~5.7 tail of
this input set, far inside the 2e-2 gate), the device streams fp16 in/out,
and the host upcasts y to fp32. All row statistics stay fp32 on device.

Device layout ("two contiguous row-chunks"): each core's [64, 32768] block is
split into 2 contiguous address-range chunks of 32 rows. Chunk k, viewed as
[128, 4096], holds rows 32k..32k+31 with row 32k+j on partitions
[4j, 4j+4) — so every DMA is one fully-contiguous 2MB transfer (measured
6-30x faster on this target than partition-interleaved patterns), and chunk
k+1's input DMA overlaps chunk k's compute while chunk k's output DMA overlaps
chunk k+1's compute. Per chunk: sigmoid as 4096-wide out-of-place ACT tiles
(fp16 in -> fp16 out) with fused fp32 row-partial accumulation (accum_out),
one PE matmul against a (1/K)-scaled 0/1 block matrix to group-sum + broadcast
the row budgets (bud_ps = budget/K; scaling by the power-of-two 1/K=2^-6 is
exact, so reciprocal(bud_ps) is bit-identical to K*reciprocal(budget) and one
DVE op cheaper), then one full-width fp16 DVE tensor_scalar scale pass (packed
4x mode; the fp32 per-partition scalar is exempt from the packing rules);
column 0 of each row is then zeroed by an in-place width-1 multiply with a
per-partition 0/1 mask to implement y[:, 0] = 0.

Math: y0 = sigmoid(scores/temp); budget_r = clip(sum_i y0[r,i], 1e-6);
y = y0 * min(K/budget, 1); then R=4 damping iters
y *= min(2/(1+y+roll(y,-d)), 1); y[:,0] = 0.

Damping-identity property (load-bearing): if budget_r >= 2K = 128 for every
row, then min(K/budget,1) <= 0.5 (correctly-rounded fp32 div), so every
y <= 0.5, so s = fl(y[i]+y[i+d]) <= 1, fl(1+s) <= 2, fl(2/(1+s)) >= 1, and
min(2/(1+s), 1.0) == 1.0 *exactly*; y*1.0 is bitwise identity. Inductively the
whole damping loop is an exact fp32 no-op (and an exact fp16 no-op, by the
same argument at any precision). For N(0,1)-like scores,
budget ~ T/2 = 16384 (margin ~128x over the threshold). The device exports the
raw per-row sums (scaled by 1/K); the host checks budget >= 256 for every row
and otherwise falls back to a full numpy evaluation of the reference semantics
(exact for arbitrary inputs; never taken for the spec'd input distribution).
The same check makes clip(budget, 1e-6) and min(K/budget, 1) identities on the
device path, so the device computes g = K * reciprocal(sum) directly.
"""

import numpy as np

B, T = 512, 32768
K = 64.0
R_REFRACTORY = 4
N_CORES = 8
ROWS = B // N_CORES  # 64 rows per core
P = 128

NCHUNK = 2
RPC = ROWS // NCHUNK  # 32 rows per chunk
GS = P // RPC  # 4 partitions per row within a chunk
WC = RPC * T // P  # 8192 free width per chunk
ACT_W = 4096  # ACT tile width

_NC_CACHE: dict = {}


def _build_nc(inv_temp: float, reps: int = 1):
    from contextlib import ExitStack

    import concourse.bacc as bacc
    import concourse.tile as tile
    from concourse import mybir

    f32 = mybir.dt.float32
    f16 = mybir.dt.float16
    nc = bacc.Bacc(
        "TRN2",
        target_bir_lowering=False,
        debug=False,
        enable_asserts=False,
        num_devices=N_CORES,
    )
    scores_h = nc.dram_tensor("scores", [ROWS, T], f16, kind="ExternalInput")
    wsum_h = nc.dram_tensor("wsum", [P, P], f32, kind="ExternalInput")
    mask_h = nc.dram_tensor("mask", [P, 1], f16, kind="ExternalInput")
    y_h = nc.dram_tensor("y", [ROWS, T], f16, kind="ExternalOutput")
    bud_h = nc.dram_tensor("budgets", [NCHUNK, P], f32, kind="ExternalOutput")

    # [nchunk, 128, Wc] flat-contiguous chunk views
    s_k = scores_h.rearrange("r (q w) -> (r q) w", w=WC).rearrange(
        "(k p) w -> k p w", p=P
    )
    y_k = y_h.rearrange("r (q w) -> (r q) w", w=WC).rearrange("(k p) w -> k p w", p=P)

    with tile.TileContext(nc) as tc, ExitStack() as ctx:
        inp = ctx.enter_context(tc.tile_pool(name="inp", bufs=2))
        sig = ctx.enter_context(tc.tile_pool(name="sig", bufs=2))
        outp = ctx.enter_context(tc.tile_pool(name="outp", bufs=2))
        stats = ctx.enter_context(tc.tile_pool(name="stats", bufs=4))
        consts = ctx.enter_context(tc.tile_pool(name="consts", bufs=1))
        psum = ctx.enter_context(tc.tile_pool(name="psum", bufs=4, space="PSUM"))

        wsum_t = consts.tile([P, P], f32)
        nc.sync.dma_start(wsum_t[:], wsum_h[:, :])
        mask_t = consts.tile([P, 1], f16)
        nc.sync.dma_start(mask_t[:], mask_h[:, :])
        # Load the sigmoid ACT table set while the first big DMA streams.
        wtile = consts.tile([P, 1], f32)
        nc.vector.memset(wtile[:], 0.0)
        nc.scalar.activation(wtile[:], wtile[:], mybir.ActivationFunctionType.Sigmoid)

        for _rep in range(reps):
            for k in range(NCHUNK):
                t_in = inp.tile([P, WC], f16, tag="in")
                nc.sync.dma_start(t_in[:], s_k[k, :, :])
                t_sig = sig.tile([P, WC], f16, tag="sig")
                ntile = WC // ACT_W
                partials = stats.tile([P, ntile], f32, tag="partials")
                for i in range(ntile):
                    sl = slice(i * ACT_W, (i + 1) * ACT_W)
                    nc.scalar.activation(
                        t_sig[:, sl],
                        t_in[:, sl],
                        mybir.ActivationFunctionType.Sigmoid,
                        scale=float(inv_temp),
                        accum_out=partials[:, i : i + 1],
                    )
                total = stats.tile([P, 1], f32, tag="total")
                nc.vector.tensor_reduce(
                    total[:],
                    partials[:],
                    axis=mybir.AxisListType.X,
                    op=mybir.AluOpType.add,
                )
                # group-sum + broadcast: bud_ps[p] = (1/K) * sum of total over
                # p's 4-group, so rb below is directly g = K/budget.
                bud_ps = psum.tile([P, 1], f32, tag="budps")
                nc.tensor.matmul(
                    bud_ps[:], wsum_t[:], total[:, 0:1], start=True, stop=True
                )
                rb = stats.tile([P, 1], f32, tag="rb")
                nc.vector.reciprocal(rb[:], bud_ps[:])
                t_out = outp.tile([P, WC], f16, tag="out")
                # fp16 in/out dense SBUF keeps the packed DVE fast path; the
                # per-partition fp32 scalar rb does not break it
                nc.vector.tensor_scalar_mul(t_out[:, :], t_sig[:, :], rb[:, 0:1])
                nc.vector.tensor_mul(t_out[:, 0:1], t_out[:, 0:1], mask_t[:, 0:1])
                nc.sync.dma_start(y_k[k, :, :], t_out[:])
                # export raw row sums (off critical path)
                bud = stats.tile([P, 1], f32, tag="bud")
                nc.vector.tensor_copy(bud[:], bud_ps[:])
                nc.gpsimd.dma_start(bud_h[k : k + 1, :], bud[:, 0:1])
    nc.compile()
    return nc


def _get_nc(inv_temp: float, reps: int = 1):
    key = (round(float(inv_temp), 9), reps)
    if key not in _NC_CACHE:
        _NC_CACHE[key] = _build_nc(inv_temp, reps)
    return _NC_CACHE[key]


def _wsum_matrix() -> np.ndarray:
    # wsum[k, m] = 1/K iff k//GS == m//GS: sums each row's GS partitions,
    # broadcasts back to all of them, and folds in the exact 2^-6 = 1/K scale
    # — one matmul does the whole reduction + scale.
    return np.kron(
        np.eye(P // GS, dtype=np.float32),
        np.full((GS, GS), 1.0 / K, dtype=np.float32),
    )


def _mask_matrix() -> np.ndarray:
    # 0 at partitions holding a row start (p % GS == 0), else 1
    m = np.ones((P, 1), np.float16)
    m[0::GS, 0] = 0.0
    return m


def make_in_maps(scores: np.ndarray) -> list:
    scores16 = np.ascontiguousarray(scores.astype(np.float16))
    wsum = _wsum_matrix()
    mask = _mask_matrix()
    return [
        {"scores": scores16[c * ROWS : (c + 1) * ROWS], "wsum": wsum, "mask": mask}
        for c in range(N_CORES)
    ]


def _temp_from_log(log_temperature) -> np.float32:
    lt = np.float32(np.asarray(log_temperature, dtype=np.float32).reshape(()))
    return np.float32(np.clip(np.exp(lt, dtype=np.float32), 0.1, 10.0))


def _reference_fallback(scores: np.ndarray, temp: np.float32) -> np.ndarray:
    # Exact general-case evaluation (mirrors reference.py in fp32 numpy).
    y = 1.0 / (1.0 + np.exp(-(scores / temp), dtype=np.float32))
    y = y.astype(np.float32)
    budget = np.clip(np.sum(y, axis=1, keepdims=True, dtype=np.float32), 1e-6, None)
    y = y * np.minimum(np.float32(K) / budget, np.float32(1.0))
    t = scores.shape[1]
    for d in range(1, min(R_REFRACTORY + 1, t)):
        shift = np.roll(y, -d, axis=1)
        y = y * np.minimum(2.0 / (1.0 + y + shift), 1.0).astype(np.float32)
    y = y.astype(np.float32)
    y[:, 0] = 0.0
    return y


def kernel(scores: np.ndarray, log_temperature: np.ndarray) -> np.ndarray:
    from concourse.bass_utils import run_bass_kernel_spmd

    scores = np.ascontiguousarray(scores, dtype=np.float32)
    assert scores.shape == (B, T), scores.shape
    temp = _temp_from_log(log_temperature)
    inv_temp = np.float32(1.0) / temp

    nc = _get_nc(float(inv_temp))
    in_maps = make_in_maps(scores)
    res = run_bass_kernel_spmd(nc, in_maps, list(range(N_CORES))).results
    y = np.concatenate([res[c]["y"] for c in range(N_CORES)], axis=0).astype(
        np.float32
    )
    # budgets[k, GS*j] = (1/K) * raw sum of row RPC*k + j (per core)
    budgets = np.concatenate(
        [res[c]["budgets"][:, 0::GS].reshape(-1) for c in range(N_CORES)]
    )

    # Damping is an exact identity iff every row budget >= 2K (see module
    # docstring); 256 adds 2x margin over the required 128 (budgets are
    # exported pre-scaled by 1/K = 1/64, hence the 4.0). If violated (never,
    # for randn-scale inputs), recompute everything faithfully on the host.
    if not np.all(budgets >= 4.0):
        return _reference_fallback(scores, temp)
    return y


# revision 4
# speedup vs baseline: 2.5537x; 2.2152x over previous
"""Trainium2 Bass kernel: nn_DifferentiableSelector (soft top-K w/ refractory damping).

Data-parallel over batch: 512 rows -> 64 rows/core on 8 NeuronCores.

Memory-regime kernel: device I/O is fp16 (half the HBM traffic of fp32; the
host rounds scores to fp16 — worst-case sigmoid rel-err ~|x|*2^-11 ~ 3e-3 at
the |x|~5.7 tail of this input set, far inside the 2e-2 gate — and upcasts y
back to fp32). All row statistics stay fp32 on device. Measured on this
target: pure fp16 streaming (in+out) runs at 19.2us/rep (~436 GB/s/core), the
fp16 sigmoid pass ~8.7us/rep (ACT does ~2 fp16 elem/cycle), DVE scale ~4.4us
(packed 4x mode) — so the kernel is latency-bound unless the per-chunk
dependency chains overlap. The structure below exists to make them overlap:

 - 4 chunks of 16 rows, each a contiguous 1MB HBM range viewed as [128, 4096]
   (row 16k+j on partitions [8j, 8j+8)): every DMA is one flat contiguous
   transfer (measured 6-30x faster here than partition-interleaved patterns).
 - All 4 input DMAs issue back-to-back from the SP sequencer at the top of
   each rep — an output DMA's semaphore wait can never head-of-line-block the
   input stream (HWDGE rings drain FIFO per issuing engine).
 - Output DMAs issue from the DVE sequencer (the other HWDGE ring) right
   after the in-place scale, whose operands are already resolved — no stall.
 - Per chunk: one full-width sigmoid ACT op (fp16 in -> fp16 out) whose fused
   fp32 accum_out IS the chunk row-partial (no separate reduce), one PE
   matmul against a (1/K)-scaled 0/1 block matrix to group-sum + broadcast
   row budgets (bud_ps = budget/K; the power-of-two 1/K=2^-6 scale is exact,
   so reciprocal(bud_ps) is bit-identical to K*reciprocal(budget)), DVE
   reciprocal, then one in-place full-width DVE tensor_scalar multiply (fp16
   operands keep the packed 4x path; the fp32 per-partition scalar is exempt
   from the packing rules).
 - y[:, 0] = 0 is applied on the host after the fp32 upcast (free) instead of
   on-device, keeping the scale -> out-DMA handoff dependency-free.
 - Budgets for all 4 chunks batch into one [P, 4] tile and export once per
   rep via the Pool/SWDGE queue (off the critical path).

Math: y0 = sigmoid(scores/temp); budget_r = clip(sum_i y0[r,i], 1e-6);
y = y0 * min(K/budget, 1); then R=4 damping iters
y *= min(2/(1+y+roll(y,-d)), 1); y[:,0] = 0.

Damping-identity property (load-bearing): if budget_r >= 2K = 128 for every
row, then min(K/budget,1) <= 0.5 (correctly-rounded fp32 div), so every
y <= 0.5, so s = fl(y[i]+y[i+d]) <= 1, fl(1+s) <= 2, fl(2/(1+s)) >= 1, and
min(2/(1+s), 1.0) == 1.0 *exactly*; y*1.0 is bitwise identity. Inductively the
whole damping loop is an exact no-op at any precision. For N(0,1)-like
scores, budget ~ T/2 = 16384 (margin ~128x over the threshold). The device
exports every row's budget (scaled by 1/K); the host checks budget >= 256
(i.e. 4.0 scaled) for every row and otherwise falls back to a full numpy
evaluation of the reference semantics (exact for arbitrary inputs; never
taken for the spec'd input distribution). The same check makes
clip(budget, 1e-6) and min(K/budget, 1) identities on the device path, so the
device computes g = K * reciprocal(sum) directly.
"""

import numpy as np

B, T = 512, 32768
K = 64.0
R_REFRACTORY = 4
N_CORES = 8
ROWS = B // N_CORES  # 64 rows per core
P = 128

NCHUNK = 4
RPC = ROWS // NCHUNK  # 16 rows per chunk
GS = P // RPC  # 8 partitions per row within a chunk
WC = RPC * T // P  # 4096 free width per chunk

_NC_CACHE: dict = {}


def _build_nc(inv_temp: float, reps: int = 1, nchunk: int = NCHUNK):
    from contextlib import ExitStack

    import concourse.bacc as bacc
    import concourse.tile as tile
    from concourse import mybir

    f32 = mybir.dt.float32
    f16 = mybir.dt.float16
    wc = RPC * T // P * NCHUNK // nchunk  # free width per chunk
    nc = bacc.Bacc(
        "TRN2",
        target_bir_lowering=False,
        debug=False,
        enable_asserts=False,
        num_devices=N_CORES,
    )
    scores_h = nc.dram_tensor("scores", [ROWS, T], f16, kind="ExternalInput")
    wsum_h = nc.dram_tensor("wsum", [P, P], f32, kind="ExternalInput")
    y_h = nc.dram_tensor("y", [ROWS, T], f16, kind="ExternalOutput")
    bud_h = nc.dram_tensor("budgets", [P, nchunk], f32, kind="ExternalOutput")

    # [nchunk, 128, wc] flat-contiguous chunk views
    s_k = scores_h.rearrange("r (q w) -> (r q) w", w=wc).rearrange(
        "(k p) w -> k p w", p=P
    )
    y_k = y_h.rearrange("r (q w) -> (r q) w", w=wc).rearrange("(k p) w -> k p w", p=P)

    with tile.TileContext(nc) as tc, ExitStack() as ctx:
        inp = ctx.enter_context(tc.tile_pool(name="inp", bufs=4))
        sig = ctx.enter_context(tc.tile_pool(name="sig", bufs=4))
        stats = ctx.enter_context(tc.tile_pool(name="stats", bufs=2 * nchunk))
        consts = ctx.enter_context(tc.tile_pool(name="consts", bufs=1))
        psum = ctx.enter_context(tc.tile_pool(name="psum", bufs=4, space="PSUM"))

        wsum_t = consts.tile([P, P], f32)
        nc.sync.dma_start(wsum_t[:], wsum_h[:, :])
        # Load the sigmoid ACT table set while the first big DMA streams.
        wtile = consts.tile([P, 1], f32)
        nc.vector.memset(wtile[:], 0.0)
        nc.scalar.activation(wtile[:], wtile[:], mybir.ActivationFunctionType.Sigmoid)

        for _rep in range(reps):
            # input stream first: nothing below can block these issues
            t_ins = []
            for k in range(nchunk):
                t_in = inp.tile([P, wc], f16, tag="in")
                nc.sync.dma_start(t_in[:], s_k[k, :, :])
                t_ins.append(t_in)
            buds = stats.tile([P, nchunk], f32, tag="buds")
            t_sigs = []
            for k in range(nchunk):
                t_sig = sig.tile([P, wc], f16, tag="sig")
                t_sigs.append(t_sig)
                total = stats.tile([P, 1], f32, tag="total")
                nc.scalar.activation(
                    t_sig[:],
                    t_ins[k][:],
                    mybir.ActivationFunctionType.Sigmoid,
                    scale=float(inv_temp),
                    accum_out=total[:],
                )
                # out-DMAs ride the Activation HWDGE ring (SP's ring stays a
                # pure input stream). Emitting out_{k-1} *after* ACT_k means
                # its scale landed ~a full ACT earlier: the ACT sequencer
                # never blocks on the wait, so sigmoids keep streaming.
                if k > 0:
                    nc.scalar.dma_start(y_k[k - 1, :, :], t_sigs[k - 1][:])
                # group-sum + broadcast: bud_ps[p] = (1/K) * sum of total over
                # p's row-group, so rb below is directly g = K/budget.
                bud_ps = psum.tile([P, 1], f32, tag="budps")
                nc.tensor.matmul(
                    bud_ps[:], wsum_t[:], total[:, 0:1], start=True, stop=True
                )
                rb = stats.tile([P, 1], f32, tag="rb")
                nc.vector.reciprocal(rb[:], bud_ps[:])
                # in-place fp16 scale keeps the packed DVE fast path; the fp32
                # per-partition scalar rb does not break it
                nc.vector.tensor_scalar_mul(t_sig[:], t_sig[:], rb[:, 0:1])
                nc.vector.tensor_copy(buds[:, k : k + 1], bud_ps[:])
            nc.scalar.dma_start(y_k[nchunk - 1, :, :], t_sigs[nchunk - 1][:])
            # one batched per-rep export, off the critical path (SWDGE/Pool)
            nc.gpsimd.dma_start(bud_h[:, :], buds[:])
    nc.compile()
    return nc


def _get_nc(inv_temp: float, reps: int = 1, nchunk: int = NCHUNK):
    key = (round(float(inv_temp), 9), reps, nchunk)
    if key not in _NC_CACHE:
        _NC_CACHE[key] = _build_nc(inv_temp, reps, nchunk)
    return _NC_CACHE[key]


def _wsum_matrix(nchunk: int = NCHUNK) -> np.ndarray:
    # wsum[k, m] = 1/K iff k//gs == m//gs: sums each row's gs partitions,
    # broadcasts back to all of them, and folds in the exact 2^-6 = 1/K scale
    # — one matmul does the whole reduction + scale.
    gs = P * nchunk // ROWS
    return np.kron(
        np.eye(P // gs, dtype=np.float32),
        np.full((gs, gs), 1.0 / K, dtype=np.float32),
    )


def make_in_maps(scores: np.ndarray, nchunk: int = NCHUNK) -> list:
    scores16 = np.ascontiguousarray(scores.astype(np.float16))
    wsum = _wsum_matrix(nchunk)
    return [
        {"scores": scores16[c * ROWS : (c + 1) * ROWS], "wsum": wsum}
        for c in range(N_CORES)
    ]


def _temp_from_log(log_temperature) -> np.float32:
    lt = np.float32(np.asarray(log_temperature, dtype=np.float32).reshape(()))
    return np.float32(np.clip(np.exp(lt, dtype=np.float32), 0.1, 10.0))


def _reference_fallback(scores: np.ndarray, temp: np.float32) -> np.ndarray:
    # Exact general-case evaluation (mirrors reference.py in fp32 numpy).
    y = 1.0 / (1.0 + np.exp(-(scores / temp), dtype=np.float32))
    y = y.astype(np.float32)
    budget = np.clip(np.sum(y, axis=1, keepdims=True, dtype=np.float32), 1e-6, None)
    y = y * np.minimum(np.float32(K) / budget, np.float32(1.0))
    t = scores.shape[1]
    for d in range(1, min(R_REFRACTORY + 1, t)):
        shift = np.roll(y, -d, axis=1)
        y = y * np.minimum(2.0 / (1.0 + y + shift), 1.0).astype(np.float32)
    y = y.astype(np.float32)
    y[:, 0] = 0.0
    return y


def kernel(scores: np.ndarray, log_temperature: np.ndarray) -> np.ndarray:
    from concourse.bass_utils import run_bass_kernel_spmd

    scores = np.ascontiguousarray(scores, dtype=np.float32)
    assert scores.shape == (B, T), scores.shape
    temp = _temp_from_log(log_temperature)
    inv_temp = np.float32(1.0) / temp

    nc = _get_nc(float(inv_temp))
    in_maps = make_in_maps(scores)
    res = run_bass_kernel_spmd(nc, in_maps, list(range(N_CORES))).results
    y = np.concatenate([res[c]["y"] for c in range(N_CORES)], axis=0).astype(
        np.float32
    )
    y[:, 0] = 0.0
    # every partition of budgets[:, k] holds a valid (broadcast) row budget
    budgets = np.stack([res[c]["budgets"] for c in range(N_CORES)])

    # Damping is an exact identity iff every row budget >= 2K (see module
    # docstring); 256 adds 2x margin over the required 128 (budgets are
    # exported pre-scaled by 1/K = 1/64, hence the 4.0). If violated (never,
    # for randn-scale inputs), recompute everything faithfully on the host.
    if not np.all(budgets >= 4.0):
        return _reference_fallback(scores, temp)
    return y
